# revision 6
# baseline (speedup 1.0000x reference)
"""Trainium2 Bass kernel for nn_Meta_Graph1_40114994545303 (gnn_message_passing).

Math: the reference returns only the global-node row of the GCN output.
With mask = (attribute_label > 0), star adjacency means
    out[s, :] = tanh( (sum_a mask[s,a] * attribute_feat[s,a,:]) @ W + b )
and x never reaches the output (adj[A, A] = 0).

Strategy (v2): cut HBM bytes, which bound the kernel.
  - feat rows with mask==0 are dropped on the host (row compaction); only
    ~half the feature rows ever cross HBM.
  - W is column-sharded across the 8 cores (1 MB fp16 per core instead of
    a replicated 8 MB); the tiny per-core masked-sum [32, 2048] is
    exchanged with an AllGather so every core can compute all 256 samples
    for its 256 output columns.
Layout trick: stage-1 PSUM column groups tile d interleaved by 32-wide
blocks (group n holds d with (d//32)%4 == n), so ONE blocked 32x32 DVE
transpose of the [128, 512] masked-sum yields aggT in exactly the
[d%128, k_chunk, sample] layout stage 2 needs as its stationary operand.
"""

import os
from contextlib import ExitStack

import numpy as np

import concourse.bacc as bacc
import concourse.mybir as mybir

B, A, D = 256, 32, 2048
NCORES = 8
S = B // NCORES  # 32 samples per core
P = 128
KC2 = D // P  # 16 k-chunks in stage 2 (contraction over d)
ESH = D // NCORES  # 256 output columns per core
F32 = mybir.dt.float32

COMPUTE_DTYPE = os.environ.get("GNN_KERNEL_DTYPE", "fp16")


def build_nc(kc1: int, compute_dtype: str = COMPUTE_DTYPE):
    """kc1: number of 128-row chunks of compacted (masked) feature rows."""
    cdt = {"f32": mybir.dt.float32, "bf16": mybir.dt.bfloat16, "fp16": mybir.dt.float16}[compute_dtype]
    nc = bacc.Bacc("TRN2", target_bir_lowering=False, debug=False, num_devices=NCORES)

    feat = nc.dram_tensor("feat", [kc1 * P, D], cdt, kind="ExternalInput")
    mbdt = nc.dram_tensor("mbdt", [P, kc1 * S], cdt, kind="ExternalInput")
    w = nc.dram_tensor("w", [P, KC2 * ESH], cdt, kind="ExternalInput")  # [p, (k e)]
    bias = nc.dram_tensor("bias", [1, ESH], cdt, kind="ExternalInput")
    out = nc.dram_tensor("out", [B, ESH], F32, kind="ExternalOutput")
    bounce_in = nc.dram_tensor("bnc_in", [P, 4 * P], cdt, kind="Internal")
    bounce_out = nc.dram_tensor(
        "bnc_out", [NCORES, P, 4 * P], cdt, kind="Internal", addr_space="Shared"
    )

    # feat DMA groups (chunks per dma_start)
    groups = []
    left = kc1
    while left > 0:
        g = min(2, left)
        groups.append(g)
        left -= g

    with ExitStack() as ctx:
        # d index decomposition: d = jj*128 + n*32 + dd
        feat_sb = ctx.enter_context(nc.sbuf_tensor([P, kc1, KC2, 4, 32], cdt))
        mbdt_sb = ctx.enter_context(nc.sbuf_tensor([P, kc1, S], cdt))
        w_sb = ctx.enter_context(nc.sbuf_tensor([P, KC2, ESH], cdt))
        bias_sb = ctx.enter_context(nc.sbuf_tensor([1, ESH], cdt))
        ones_sb = ctx.enter_context(nc.sbuf_tensor([1, P], cdt))
        agg_sb = ctx.enter_context(nc.sbuf_tensor([P, 4 * P], cdt))
        aggt_sb = ctx.enter_context(nc.sbuf_tensor([P, KC2, S], cdt))
        gath_sb = ctx.enter_context(nc.sbuf_tensor([P, NCORES, KC2, S], cdt))
        out_sb = ctx.enter_context(nc.sbuf_tensor([P, 2, ESH], F32))
        pm = ctx.enter_context(nc.psum_tensor([P, 4 * P], F32))
        pg0 = ctx.enter_context(nc.psum_tensor("pg0", [P, ESH], F32))
        pg1 = ctx.enter_context(nc.psum_tensor("pg1", [P, ESH], F32))
        pg = [pg0, pg1]

        fsems = [ctx.enter_context(nc.semaphore(f"fs{g}")) for g in range(len(groups))]
        wsem = ctx.enter_context(nc.semaphore("wsem"))
        csem = ctx.enter_context(nc.semaphore("csem"))
        vsem = ctx.enter_context(nc.semaphore("vsem"))
        s1_sem = ctx.enter_context(nc.semaphore("s1_sem"))
        tr_sem = ctx.enter_context(nc.semaphore("tr_sem"))
        bsem = ctx.enter_context(nc.semaphore("bsem"))
        ccsem = ctx.enter_context(nc.semaphore("ccsem"))
        rbsem = ctx.enter_context(nc.semaphore("rbsem"))
        s2_sem = ctx.enter_context(nc.semaphore("s2_sem"))
        act_sem = ctx.enter_context(nc.semaphore("act_sem"))
        osem = ctx.enter_context(nc.semaphore("osem"))
        block = ctx.enter_context(nc.Block(no_gpsimd_drain=True))

        @block.sync
        def _(sync):
            st = 0
            for g, cf in enumerate(groups):
                sync.dma_start(
                    feat_sb[:, st : st + cf, :, :, :].rearrange(
                        "p c a b e -> p c (a b e)"
                    ),
                    feat[st * P : (st + cf) * P, :].rearrange("(c p) d -> p c d", p=P),
                ).then_inc(fsems[g], 16)
                st += cf
            sync.dma_start(
                w_sb[:].rearrange("p k e -> p (k e)"), w[:]
            ).then_inc(wsem, 16)

        @block.scalar
        def _(scalar):
            scalar.dma_start(
                mbdt_sb[:], mbdt[:].rearrange("p (k j) -> p k j", k=kc1)
            ).then_inc(csem, 16)
            scalar.dma_start(bias_sb[:], bias[:]).then_inc(csem, 16)
            scalar.wait_ge(s2_sem, 1)
            last = None
            for g in range(2):
                last = nc.scalar.activation(
                    out_sb[:, g, :], pg[g][:], mybir.ActivationFunctionType.Tanh
                )
            last.then_inc(act_sem, 1)
            scalar.wait_ge(act_sem, 1)
            scalar.dma_start(
                out[:].rearrange("(g p) e -> p g e", p=P), out_sb[:]
            ).then_inc(osem, 16)
            scalar.wait_ge(osem, 16)

        @block.vector
        def _(vector):
            nc.vector.memset(ones_sb[:], 1.0).then_inc(vsem, 1)
            vector.wait_ge(s1_sem, 1)
            nc.vector.tensor_copy(agg_sb[:], pm[:])
            nc.vector.drain()
            nc.vector.transpose(
                aggt_sb[:].rearrange("p k j -> p (k j)"), agg_sb[:]
            ).then_inc(tr_sem, 1)

        @block.gpsimd
        def _(gpsimd):
            gpsimd.wait_ge(tr_sem, 1)
            gpsimd.dma_start(
                bounce_in[:], aggt_sb[:].rearrange("p k j -> p (k j)")
            ).then_inc(bsem, 16)
            gpsimd.wait_ge(bsem, 16)
            nc.gpsimd.collective_compute(
                "AllGather",
                mybir.AluOpType.bypass,
                replica_groups=[list(range(NCORES))],
                ins=[bounce_in[:]],
                outs=[bounce_out[:]],
            ).then_inc(ccsem, 1)
            gpsimd.wait_ge(ccsem, 1)
            gpsimd.dma_start(
                gath_sb[:].rearrange("p c k j -> p c (k j)"),
                bounce_out[:].rearrange("c p f -> p c f"),
            ).then_inc(rbsem, 16)

        @block.tensor
        def _(tensor):
            tensor.wait_ge(csem, 16)  # mbdt resident
            last = None
            st = 0
            for g, cf in enumerate(groups):
                tensor.wait_ge(fsems[g], 16)
                for ci in range(cf):
                    kc = st + ci
                    for n in range(4):
                        last = nc.tensor.matmul(
                            pm[n * S : (n + 1) * S, :],
                            mbdt_sb[:, kc, :],
                            feat_sb[:, kc, :, n, :],
                            start=(kc == 0),
                            stop=(kc == kc1 - 1),
                            tile_position=(0, n * S),
                            skip_group_check=True,
                        )
                st += cf
            last.then_inc(s1_sem, 1)

            # stage 2: bias first (opens accumulation), then 16 k-chunks x 8 cores
            tensor.wait_ge(csem, 32)
            tensor.wait_ge(vsem, 1)
            for g in range(2):
                nc.tensor.matmul(
                    pg[g][:],
                    ones_sb[:],
                    bias_sb[:],
                    start=True,
                    stop=False,
                    tile_position=(0, 0),
                    skip_group_check=True,
                )
            tensor.wait_ge(wsem, 16)
            tensor.wait_ge(rbsem, 16)
            lastb = None
            for k in range(KC2):
                for c in range(NCORES):
                    g, u = divmod(c, 4)
                    lastb = nc.tensor.matmul(
                        pg[g][u * S : (u + 1) * S, :],
                        gath_sb[:, c, k, :],
                        w_sb[:, k, :],
                        start=False,
                        stop=(k == KC2 - 1),
                        tile_position=(0, u * S),
                        skip_group_check=True,
                    )
            lastb.then_inc(s2_sem, 1)

    nc.compile()
    return nc


def _host_prep(inputs: dict, compute_dtype: str):
    np_cdt = {"f32": np.float32, "bf16": None, "fp16": np.float16}[compute_dtype]
    if np_cdt is None:
        import ml_dtypes

        np_cdt = ml_dtypes.bfloat16

    feat = np.asarray(inputs["attribute_feat"])
    label = np.asarray(inputs["attribute_label"])
    w = np.asarray(inputs["W"], dtype=np.float32)
    b = np.asarray(inputs["b"], dtype=np.float32).reshape(1, D)
    mask = label > 0  # [B, A] bool

    # per-core compacted row counts -> common padded chunk count
    counts = [int(mask[c * S : (c + 1) * S].sum()) for c in range(NCORES)]
    kc1 = max(1, (max(counts) + P - 1) // P)
    rpad = kc1 * P

    in_maps = []
    for c in range(NCORES):
        m_c = mask[c * S : (c + 1) * S]  # [S, A]
        feat_c = feat[c * S : (c + 1) * S]  # [S, A, D]
        s_idx, a_idx = np.nonzero(m_c)  # row-major: sorted by sample
        r = len(s_idx)
        fc = np.zeros((rpad, D), np_cdt)
        fc[:r] = feat_c[s_idx, a_idx].astype(np_cdt)
        mbd = np.zeros((P, kc1, S), np.float32)
        rows = np.arange(r)
        mbd[rows % P, rows // P, s_idx] = 1.0
        wc = (
            w[:, c * ESH : (c + 1) * ESH]
            .reshape(KC2, P, ESH)
            .transpose(1, 0, 2)
            .reshape(P, KC2 * ESH)
        )
        in_maps.append(
            {
                "feat": fc,
                "mbdt": mbd.reshape(P, kc1 * S).astype(np_cdt),
                "w": np.ascontiguousarray(wc).astype(np_cdt),
                "bias": b.astype(np_cdt),
            }
        )
    return in_maps, kc1


_NC_CACHE: dict = {}


def run(inputs: dict, compute_dtype: str = COMPUTE_DTYPE, trace: bool = False):
    from concourse.bass_utils import run_bass_kernel_spmd

    in_maps, kc1 = _host_prep(inputs, compute_dtype)
    key = (compute_dtype, kc1)
    if key not in _NC_CACHE:
        _NC_CACHE[key] = build_nc(kc1, compute_dtype)
    nc = _NC_CACHE[key]
    res = run_bass_kernel_spmd(nc, in_maps, list(range(NCORES)), trace=trace)
    out = np.concatenate([res.results[c]["out"] for c in range(NCORES)], axis=1)
    return out, res


def kernel(**inputs) -> np.ndarray:
    out, _ = run(inputs)
    return out


# revision 16
# speedup vs baseline: 2.0690x; 2.0690x over previous
"""Trainium2 Bass kernel for nn_Meta_Graph1_40114994545303 (gnn_message_passing).

Math: the reference returns only the global-node row of the GCN output.
With mask = (attribute_label > 0), star adjacency means
    out[s, :] = tanh( (sum_a mask[s,a] * attribute_feat[s,a,:]) @ W + b )
and x never reaches the output (adj[A, A] = 0).

Strategy (v3): data-parallel over batch (32 samples/core), no collectives
(NRT AllGather costs ~65us fixed here). HBM bytes minimized + spread over
multiple DMA queues:
  - feat rows with mask==0 are dropped on the host (row compaction), ~2.4MB
    fp16 per core instead of 4MB, streamed on the sync queue.
  - W (8MB fp16, replicated) split across vector+gpsimd+tensor queues in
    parallel with feat.
Stage-1 PSUM column groups tile d interleaved by 32-wide blocks (group n
holds d with (d//32)%4 == n), so ONE blocked 32x32 DVE transpose of the
[128, 512] masked-sum yields aggT in the [d%128, k_chunk, sample] layout
stage 2 needs as its stationary operand (replaces 64 DVE transposes).
"""

import os
from contextlib import ExitStack

import numpy as np

import concourse.bacc as bacc
import concourse.mybir as mybir

B, A, D = 256, 32, 2048
NCORES = 8
S = B // NCORES  # 32 samples per core
P = 128
KC2 = D // P  # 16 k-chunks in stage 2 (contraction over d)
NT = D // 512  # 4 psum-bank-wide output column tiles
F32 = mybir.dt.float32

COMPUTE_DTYPE = os.environ.get("GNN_KERNEL_DTYPE", "fp16")


def build_nc(kc1: int, compute_dtype: str = COMPUTE_DTYPE):
    """kc1: number of 128-row chunks of compacted (masked) feature rows."""
    cdt = {"f32": mybir.dt.float32, "bf16": mybir.dt.bfloat16, "fp16": mybir.dt.float16}[compute_dtype]
    nc = bacc.Bacc("TRN2", target_bir_lowering=False, debug=False)

    feat = nc.dram_tensor("feat", [kc1 * P, D], cdt, kind="ExternalInput")
    mbdt = nc.dram_tensor("mbdt", [P, kc1 * S], cdt, kind="ExternalInput")
    w = nc.dram_tensor("w", [P, KC2 * D], cdt, kind="ExternalInput")  # [p, (k e)]
    bias = nc.dram_tensor("bias", [1, D], cdt, kind="ExternalInput")
    out = nc.dram_tensor("out", [S, D], F32, kind="ExternalOutput")

    # feat DMA groups (chunks per dma_start)
    groups = []
    left = kc1
    while left > 0:
        g = min(2, left)
        groups.append(g)
        left -= g
    # W k-chunk DMA splits: (queue, start, len) — a=scalar, g=gpsimd, s=sync
    wsplit = [("a", 0, 3), ("g", 3, 3), ("a", 6, 3), ("g", 9, 3), ("s", 12, 2), ("s", 14, 2)]

    with ExitStack() as ctx:
        # d index decomposition: d = jj*128 + n*32 + dd
        feat_sb = ctx.enter_context(nc.sbuf_tensor([P, kc1, KC2, 4, 32], cdt))
        mbdt_sb = ctx.enter_context(nc.sbuf_tensor([P, kc1, S], cdt))
        w_sb = ctx.enter_context(nc.sbuf_tensor([P, KC2, D], cdt))
        bias_sb = ctx.enter_context(nc.sbuf_tensor([1, D], cdt))
        ones_sb = ctx.enter_context(nc.sbuf_tensor([1, S], cdt))
        agg_sb = ctx.enter_context(nc.sbuf_tensor([P, 4 * P], cdt))
        aggt_sb = ctx.enter_context(nc.sbuf_tensor([P, KC2, S], cdt))
        out_sb = ctx.enter_context(nc.sbuf_tensor([P, 512], F32))
        pm = ctx.enter_context(nc.psum_tensor([P, 4 * P], F32))
        po = ctx.enter_context(nc.psum_tensor([P, 512], F32))

        fsems = [ctx.enter_context(nc.semaphore(f"fs{g}")) for g in range(len(groups))]
        wsems = [ctx.enter_context(nc.semaphore(f"ws{g}")) for g in range(len(wsplit))]
        csem = ctx.enter_context(nc.semaphore("csem"))
        vsem = ctx.enter_context(nc.semaphore("vsem"))
        s1_sem = ctx.enter_context(nc.semaphore("s1_sem"))
        tr_sem = ctx.enter_context(nc.semaphore("tr_sem"))
        s2_sem = ctx.enter_context(nc.semaphore("s2_sem"))
        act_sem = ctx.enter_context(nc.semaphore("act_sem"))
        osem = ctx.enter_context(nc.semaphore("osem"))
        block = ctx.enter_context(nc.Block(no_gpsimd_drain=True))

        def w_dma(eng, i):
            _, st, ln = wsplit[i]
            eng.dma_start(
                w_sb[:, st : st + ln, :].rearrange("p k e -> p (k e)"),
                w[:, st * D : (st + ln) * D],
            ).then_inc(wsems[i], 16)

        @block.sync
        def _(sync):
            st = 0
            for g, cf in enumerate(groups):
                sync.dma_start(
                    feat_sb[:, st : st + cf, :, :, :].rearrange(
                        "p c a b e -> p c (a b e)"
                    ),
                    feat[st * P : (st + cf) * P, :].rearrange("(c p) d -> p c d", p=P),
                ).then_inc(fsems[g], 16)
                st += cf
            for i, (q, _, _) in enumerate(wsplit):
                if q == "s":
                    w_dma(sync, i)
            sync.wait_ge(act_sem, 1)
            for n in (0, 2):
                sync.dma_start(
                    out[:, n * 512 : (n + 1) * 512], out_sb[n * S : (n + 1) * S, :]
                ).then_inc(osem, 16)
            sync.wait_ge(osem, 64)

        @block.scalar
        def _(scalar):
            scalar.dma_start(
                mbdt_sb[:], mbdt[:].rearrange("p (k j) -> p k j", k=kc1)
            ).then_inc(csem, 16)
            scalar.dma_start(bias_sb[:], bias[:]).then_inc(csem, 16)
            for i, (q, _, _) in enumerate(wsplit):
                if q == "a":
                    w_dma(scalar, i)
            scalar.wait_ge(s2_sem, 1)
            nc.scalar.activation(
                out_sb[:], po[:], mybir.ActivationFunctionType.Tanh
            ).then_inc(act_sem, 1)
            scalar.wait_ge(act_sem, 1)
            for n in (1, 3):
                scalar.dma_start(
                    out[:, n * 512 : (n + 1) * 512], out_sb[n * S : (n + 1) * S, :]
                ).then_inc(osem, 16)
            scalar.wait_ge(osem, 64)

        @block.vector
        def _(vector):
            nc.vector.memset(ones_sb[:], 1.0).then_inc(vsem, 1)
            vector.wait_ge(s1_sem, 1)
            nc.vector.tensor_copy(agg_sb[:], pm[:])
            nc.vector.drain()
            nc.vector.transpose(
                aggt_sb[:].rearrange("p k j -> p (k j)"), agg_sb[:]
            ).then_inc(tr_sem, 1)

        @block.gpsimd
        def _(gpsimd):
            for i, (q, _, _) in enumerate(wsplit):
                if q == "g":
                    w_dma(gpsimd, i)

        @block.tensor
        def _(tensor):
            tensor.wait_ge(csem, 16)  # mbdt resident
            last = None
            st = 0
            for g, cf in enumerate(groups):
                tensor.wait_ge(fsems[g], 16)
                for ci in range(cf):
                    kc = st + ci
                    for n in range(4):
                        last = nc.tensor.matmul(
                            pm[n * S : (n + 1) * S, :],
                            mbdt_sb[:, kc, :],
                            feat_sb[:, kc, :, n, :],
                            start=(kc == 0),
                            stop=(kc == kc1 - 1),
                            tile_position=(0, n * S),
                            skip_group_check=True,
                        )
                st += cf
            last.then_inc(s1_sem, 1)

            # stage 2: bias first (opens accumulation) as rank-1 matmul
            tensor.wait_ge(csem, 32)
            tensor.wait_ge(vsem, 1)
            for n in range(NT):
                nc.tensor.matmul(
                    po[n * S : (n + 1) * S, :],
                    ones_sb[:],
                    bias_sb[:, n * 512 : (n + 1) * 512],
                    start=True,
                    stop=False,
                    tile_position=(0, n * S),
                    skip_group_check=True,
                )
            tensor.wait_ge(tr_sem, 1)
            lastb = None
            for i, (_, st2, ln) in enumerate(wsplit):
                tensor.wait_ge(wsems[i], 16)
                for kk in range(ln):
                    k = st2 + kk
                    for n in range(NT):
                        lastb = nc.tensor.matmul(
                            po[n * S : (n + 1) * S, :],
                            aggt_sb[:, k, :],
                            w_sb[:, k, n * 512 : (n + 1) * 512],
                            start=False,
                            stop=(k == KC2 - 1),
                            tile_position=(0, n * S),
                            skip_group_check=True,
                        )
            lastb.then_inc(s2_sem, 1)

    nc.compile()
    return nc


def _host_prep(inputs: dict, compute_dtype: str):
    np_cdt = {"f32": np.float32, "bf16": None, "fp16": np.float16}[compute_dtype]
    if np_cdt is None:
        import ml_dtypes

        np_cdt = ml_dtypes.bfloat16

    feat = np.asarray(inputs["attribute_feat"])
    label = np.asarray(inputs["attribute_label"])
    w = np.asarray(inputs["W"], dtype=np.float32)
    b = np.asarray(inputs["b"], dtype=np.float32).reshape(1, D)
    mask = label > 0  # [B, A] bool

    counts = [int(mask[c * S : (c + 1) * S].sum()) for c in range(NCORES)]
    kc1 = max(1, (max(counts) + P - 1) // P)
    rpad = kc1 * P

    wp = np.ascontiguousarray(
        w.reshape(KC2, P, D).transpose(1, 0, 2).reshape(P, KC2 * D)
    ).astype(np_cdt)
    bp = b.astype(np_cdt)

    in_maps = []
    for c in range(NCORES):
        m_c = mask[c * S : (c + 1) * S]  # [S, A]
        feat_c = feat[c * S : (c + 1) * S]  # [S, A, D]
        s_idx, a_idx = np.nonzero(m_c)  # row-major: sorted by sample
        r = len(s_idx)
        fc = np.zeros((rpad, D), np_cdt)
        fc[:r] = feat_c[s_idx, a_idx].astype(np_cdt)
        mbd = np.zeros((P, kc1, S), np.float32)
        rows = np.arange(r)
        mbd[rows % P, rows // P, s_idx] = 1.0
        in_maps.append(
            {
                "feat": fc,
                "mbdt": mbd.reshape(P, kc1 * S).astype(np_cdt),
                "w": wp,
                "bias": bp,
            }
        )
    return in_maps, kc1


_NC_CACHE: dict = {}


def run(inputs: dict, compute_dtype: str = COMPUTE_DTYPE, trace: bool = False):
    from concourse.bass_utils import run_bass_kernel_spmd

    in_maps, kc1 = _host_prep(inputs, compute_dtype)
    key = (compute_dtype, kc1)
    if key not in _NC_CACHE:
        _NC_CACHE[key] = build_nc(kc1, compute_dtype)
    nc = _NC_CACHE[key]
    res = run_bass_kernel_spmd(nc, in_maps, list(range(NCORES)), trace=trace)
    out = np.concatenate([res.results[c]["out"] for c in range(NCORES)], axis=0)
    return out, res


def kernel(**inputs) -> np.ndarray:
    out, _ = run(inputs)
    return out


# revision 19
# speedup vs baseline: 2.3096x; 1.1163x over previous
"""Trainium2 Bass kernel for nn_Meta_Graph1_40114994545303 (gnn_message_passing).

Math: the reference returns only the global-node row of the GCN output.
With mask = (attribute_label > 0), star adjacency means
    out[s, :] = tanh( (sum_a mask[s,a] * attribute_feat[s,a,:]) @ W + b )
and x never reaches the output (adj[A, A] = 0).

Strategy (v4): per-core HBM is the bottleneck (~400 GB/s shared by all
queues), collectives cost ~65us fixed here (unusable), so minimize bytes
with a 4x2 core grid: core (i, j) handles sample-group i (64 samples,
masked feature rows host-compacted, ~4.2MB fp16, replicated across j) and
W column-half j (4MB fp16). 8.5MB/core vs 12.7MB for the baseline.

Feature rows stream first at full port rate; W streams right after and
stage 2 chases its k-chunks. Stage-1 PSUM (one bank per 32-sample half)
tiles d interleaved by 32-wide blocks (group n holds d with (d//32)%4==n),
so ONE blocked 32x32 DVE transpose per half yields aggT in the
[d%128, k_chunk, sample] layout stage 2 needs as its stationary operand.
"""

import os
from contextlib import ExitStack

import numpy as np

import concourse.bacc as bacc
import concourse.mybir as mybir

B, A, D = 256, 32, 2048
NCORES = 8
GS = 4  # sample groups
ES = NCORES // GS  # 2 W column halves
SC = B // GS  # 64 samples per core
SH = SC // 2  # 32 samples per psum half
EW = D // ES  # 1024 output columns per core
P = 128
KC2 = D // P  # 16 k-chunks in stage 2 (contraction over d)
F32 = mybir.dt.float32

COMPUTE_DTYPE = os.environ.get("GNN_KERNEL_DTYPE", "fp16")


def build_nc(kc1: int, compute_dtype: str = COMPUTE_DTYPE):
    """kc1: number of 128-row chunks of compacted (masked) feature rows."""
    cdt = {"f32": mybir.dt.float32, "bf16": mybir.dt.bfloat16, "fp16": mybir.dt.float16}[compute_dtype]
    nc = bacc.Bacc("TRN2", target_bir_lowering=False, debug=False)

    feat = nc.dram_tensor("feat", [kc1 * P, D], cdt, kind="ExternalInput")
    mbdt = nc.dram_tensor("mbdt", [P, kc1 * SC], cdt, kind="ExternalInput")
    w = nc.dram_tensor("w", [P, KC2 * EW], cdt, kind="ExternalInput")  # [p,(k e)]
    bias = nc.dram_tensor("bias", [1, EW], cdt, kind="ExternalInput")
    out = nc.dram_tensor("out", [SC, EW], F32, kind="ExternalOutput")

    # feat DMA groups (chunks per dma_start)
    groups = []
    left = kc1
    while left > 0:
        g = min(2, left)
        groups.append(g)
        left -= g
    NWG = 8  # W DMA groups (2 k-chunks each)
    WGL = KC2 // NWG

    with ExitStack() as ctx:
        # d index decomposition: d = jj*128 + n*32 + dd
        feat_sb = ctx.enter_context(nc.sbuf_tensor([P, kc1, KC2, 4, 32], cdt))
        mbdt_sb = ctx.enter_context(nc.sbuf_tensor([P, kc1, 2, SH], cdt))
        w_sb = ctx.enter_context(nc.sbuf_tensor([P, KC2, EW], cdt))
        bias_sb = ctx.enter_context(nc.sbuf_tensor([1, EW], cdt))
        ones_sb = ctx.enter_context(nc.sbuf_tensor([1, SH], cdt))
        agg_sb = [
            ctx.enter_context(nc.sbuf_tensor(f"agg{h}", [P, D // 4], cdt))
            for h in range(2)
        ]
        aggt_sb = [
            ctx.enter_context(nc.sbuf_tensor(f"aggt{h}", [P, KC2, SH], cdt))
            for h in range(2)
        ]
        out_sb = ctx.enter_context(nc.sbuf_tensor([P, 512], F32))
        pm = [
            ctx.enter_context(nc.psum_tensor(f"pm{h}", [P, D // 4], F32))
            for h in range(2)
        ]
        po = ctx.enter_context(nc.psum_tensor([P, 512], F32))

        fsems = [ctx.enter_context(nc.semaphore(f"fs{g}")) for g in range(len(groups))]
        wsems = [ctx.enter_context(nc.semaphore(f"ws{g}")) for g in range(NWG)]
        csem = ctx.enter_context(nc.semaphore("csem"))
        vsem = ctx.enter_context(nc.semaphore("vsem"))
        s1_sem = [ctx.enter_context(nc.semaphore(f"s1_{h}")) for h in range(2)]
        tr_sem = ctx.enter_context(nc.semaphore("tr_sem"))
        s2_sem = ctx.enter_context(nc.semaphore("s2_sem"))
        act_sem = ctx.enter_context(nc.semaphore("act_sem"))
        osem = ctx.enter_context(nc.semaphore("osem"))
        block = ctx.enter_context(nc.Block(no_gpsimd_drain=True))

        @block.sync
        def _(sync):
            st = 0
            for g, cf in enumerate(groups):
                sync.dma_start(
                    feat_sb[:, st : st + cf, :, :, :].rearrange(
                        "p c a b e -> p c (a b e)"
                    ),
                    feat[st * P : (st + cf) * P, :].rearrange("(c p) d -> p c d", p=P),
                ).then_inc(fsems[g], 16)
                st += cf
            sync.wait_ge(act_sem, 1)
            for u in (0, 2):
                sh, et = divmod(u, 2)
                sync.dma_start(
                    out[sh * SH : (sh + 1) * SH, et * 512 : (et + 1) * 512],
                    out_sb[u * SH : (u + 1) * SH, :],
                ).then_inc(osem, 16)
            sync.wait_ge(osem, 64)

        @block.scalar
        def _(scalar):
            scalar.dma_start(
                mbdt_sb[:].rearrange("p k h j -> p (k h j)"), mbdt[:]
            ).then_inc(csem, 16)
            scalar.dma_start(bias_sb[:], bias[:]).then_inc(csem, 16)
            # W after feat: wait for the last feat group's data, then stream
            scalar.wait_ge(fsems[len(groups) - 1], 16)
            for g in range(NWG):
                st2 = g * WGL
                scalar.dma_start(
                    w_sb[:, st2 : st2 + WGL, :].rearrange("p k e -> p (k e)"),
                    w[:, st2 * EW : (st2 + WGL) * EW],
                ).then_inc(wsems[g], 16)
            scalar.wait_ge(s2_sem, 1)
            nc.scalar.activation(
                out_sb[:], po[:], mybir.ActivationFunctionType.Tanh
            ).then_inc(act_sem, 1)
            scalar.wait_ge(act_sem, 1)
            for u in (1, 3):
                sh, et = divmod(u, 2)
                scalar.dma_start(
                    out[sh * SH : (sh + 1) * SH, et * 512 : (et + 1) * 512],
                    out_sb[u * SH : (u + 1) * SH, :],
                ).then_inc(osem, 16)
            scalar.wait_ge(osem, 64)

        @block.vector
        def _(vector):
            nc.vector.memset(ones_sb[:], 1.0).then_inc(vsem, 1)
            for h in range(2):
                vector.wait_ge(s1_sem[h], 1)
                nc.vector.tensor_copy(agg_sb[h][:], pm[h][:])
                nc.vector.drain()
                nc.vector.transpose(
                    aggt_sb[h][:].rearrange("p k j -> p (k j)"), agg_sb[h][:]
                ).then_inc(tr_sem, 1)

        @block.tensor
        def _(tensor):
            tensor.wait_ge(csem, 16)  # mbdt resident
            last = [None, None]
            st = 0
            for g, cf in enumerate(groups):
                tensor.wait_ge(fsems[g], 16)
                for ci in range(cf):
                    kc = st + ci
                    for h in range(2):
                        for n in range(4):
                            last[h] = nc.tensor.matmul(
                                pm[h][n * SH : (n + 1) * SH, :],
                                mbdt_sb[:, kc, h, :],
                                feat_sb[:, kc, :, n, :],
                                start=(kc == 0),
                                stop=(kc == kc1 - 1),
                                tile_position=(0, n * SH),
                                skip_group_check=True,
                            )
                st += cf
            for h in range(2):
                last[h].then_inc(s1_sem[h], 1)

            # stage 2: bias first (opens accumulation) as rank-1 matmul
            tensor.wait_ge(csem, 32)
            tensor.wait_ge(vsem, 1)
            for u in range(4):
                sh, et = divmod(u, 2)
                nc.tensor.matmul(
                    po[u * SH : (u + 1) * SH, :],
                    ones_sb[:],
                    bias_sb[:, et * 512 : (et + 1) * 512],
                    start=True,
                    stop=False,
                    tile_position=(0, u * SH),
                    skip_group_check=True,
                )
            tensor.wait_ge(tr_sem, 2)
            lastb = None
            for g in range(NWG):
                tensor.wait_ge(wsems[g], 16)
                for kk in range(WGL):
                    k = g * WGL + kk
                    for u in range(4):
                        sh, et = divmod(u, 2)
                        lastb = nc.tensor.matmul(
                            po[u * SH : (u + 1) * SH, :],
                            aggt_sb[sh][:, k, :],
                            w_sb[:, k, et * 512 : (et + 1) * 512],
                            start=False,
                            stop=(k == KC2 - 1),
                            tile_position=(0, u * SH),
                            skip_group_check=True,
                        )
            lastb.then_inc(s2_sem, 1)

    nc.compile()
    return nc


def _host_prep(inputs: dict, compute_dtype: str):
    np_cdt = {"f32": np.float32, "bf16": None, "fp16": np.float16}[compute_dtype]
    if np_cdt is None:
        import ml_dtypes

        np_cdt = ml_dtypes.bfloat16

    feat = np.asarray(inputs["attribute_feat"])
    label = np.asarray(inputs["attribute_label"])
    w = np.asarray(inputs["W"], dtype=np.float32)
    b = np.asarray(inputs["b"], dtype=np.float32).reshape(1, D)
    mask = label > 0  # [B, A] bool

    counts = [int(mask[i * SC : (i + 1) * SC].sum()) for i in range(GS)]
    kc1 = max(1, (max(counts) + P - 1) // P)
    rpad = kc1 * P

    # per sample-group feat compaction + block-diag mask
    gfeat, gmbd = [], []
    for i in range(GS):
        m_i = mask[i * SC : (i + 1) * SC]  # [SC, A]
        feat_i = feat[i * SC : (i + 1) * SC]
        s_idx, a_idx = np.nonzero(m_i)
        r = len(s_idx)
        fc = np.zeros((rpad, D), np_cdt)
        fc[:r] = feat_i[s_idx, a_idx].astype(np_cdt)
        # mbdt layout [P, kc1, 2, SH]: sample s -> (h, j) = (s // SH, s % SH)
        mbd = np.zeros((P, kc1, 2, SH), np.float32)
        rows = np.arange(r)
        mbd[rows % P, rows // P, s_idx // SH, s_idx % SH] = 1.0
        gfeat.append(fc)
        gmbd.append(mbd.reshape(P, kc1 * SC).astype(np_cdt))

    # per e-half packed W + bias
    gw, gb = [], []
    for j in range(ES):
        wj = w[:, j * EW : (j + 1) * EW]  # [D, EW]
        wp = np.ascontiguousarray(
            wj.reshape(KC2, P, EW).transpose(1, 0, 2).reshape(P, KC2 * EW)
        ).astype(np_cdt)
        gw.append(wp)
        gb.append(b[:, j * EW : (j + 1) * EW].astype(np_cdt))

    in_maps = []
    for c in range(NCORES):
        i, j = divmod(c, ES)
        in_maps.append(
            {"feat": gfeat[i], "mbdt": gmbd[i], "w": gw[j], "bias": gb[j]}
        )
    return in_maps, kc1


_NC_CACHE: dict = {}


def run(inputs: dict, compute_dtype: str = COMPUTE_DTYPE, trace: bool = False):
    from concourse.bass_utils import run_bass_kernel_spmd

    in_maps, kc1 = _host_prep(inputs, compute_dtype)
    key = (compute_dtype, kc1)
    if key not in _NC_CACHE:
        _NC_CACHE[key] = build_nc(kc1, compute_dtype)
    nc = _NC_CACHE[key]
    res = run_bass_kernel_spmd(nc, in_maps, list(range(NCORES)), trace=trace)
    rows = []
    for i in range(GS):
        rows.append(
            np.concatenate(
                [res.results[i * ES + j]["out"] for j in range(ES)], axis=1
            )
        )
    out = np.concatenate(rows, axis=0)
    return out, res


def kernel(**inputs) -> np.ndarray:
    out, _ = run(inputs)
    return out


# revision 21
# speedup vs baseline: 2.6356x; 1.1412x over previous
"""Trainium2 Bass kernel for nn_Meta_Graph1_40114994545303 (gnn_message_passing).

Math: the reference returns only the global-node row of the GCN output.
With mask = (attribute_label > 0), star adjacency means
    out[s, :] = tanh( (sum_a mask[s,a] * attribute_feat[s,a,:]) @ W + b )
and x never reaches the output (adj[A, A] = 0).

Strategy (v5): per-core HBM is the bottleneck (~420 GB/s shared by all
queues), collectives cost ~65us fixed (unusable), so minimize bytes with a
4x2 core grid: core (i, j) handles sample-group i (64 samples, masked
feature rows host-compacted ~4.3MB fp16, replicated across j) and W
column-half j (4MB fp16). ~8.6MB/core vs 12.7MB for the baseline.

Schedule: feature rows stream first at full port rate; W triggers early
enough to roll in as feat drains, streamed e-half-major so the first
output half finishes (tanh + store) while the second half still streams.
Stage 1 issues matmuls only for chunks a sample-half actually occupies
(rows are sample-sorted), so the first half's masked-sum transposes while
the second half accumulates. Stage-1 PSUM (one bank per 32-sample half)
tiles d interleaved by 32-wide blocks (group n holds d with (d//32)%4==n),
so ONE blocked 32x32 DVE transpose per half yields aggT in the
[d%128, k_chunk, sample] layout stage 2 needs as its stationary operand.
"""

import os
from contextlib import ExitStack

import numpy as np

import concourse.bacc as bacc
import concourse.mybir as mybir

B, A, D = 256, 32, 2048
NCORES = 8
GS = 4  # sample groups
ES = NCORES // GS  # 2 W column halves
SC = B // GS  # 64 samples per core
SH = SC // 2  # 32 samples per psum half
EW = D // ES  # 1024 output columns per core
P = 128
KC2 = D // P  # 16 k-chunks in stage 2 (contraction over d)
F32 = mybir.dt.float32
F16 = mybir.dt.float16

COMPUTE_DTYPE = os.environ.get("GNN_KERNEL_DTYPE", "fp16")


def build_nc(cfg, compute_dtype: str = COMPUTE_DTYPE):
    """cfg = (kc1, rtail, ch0_end, ch1_start): feat chunk count, rows in the
    final (partial) chunk, last chunk (excl) of sample-half 0, first chunk of
    sample-half 1."""
    kc1, rtail, ch0_end, ch1_start = cfg
    cdt = {"f32": mybir.dt.float32, "bf16": mybir.dt.bfloat16, "fp16": mybir.dt.float16}[compute_dtype]
    nc = bacc.Bacc("TRN2", target_bir_lowering=False, debug=False)

    rtot = (kc1 - 1) * P + rtail
    feat = nc.dram_tensor("feat", [rtot, D], cdt, kind="ExternalInput")
    mbdt = nc.dram_tensor("mbdt", [P, kc1 * SC], cdt, kind="ExternalInput")
    w = nc.dram_tensor("w", [P, ES * KC2 * 512], cdt, kind="ExternalInput")
    bias = nc.dram_tensor("bias", [1, EW], cdt, kind="ExternalInput")
    out = nc.dram_tensor("out", [SC, EW], cdt, kind="ExternalOutput")

    # feat DMA groups of full chunks, singleton at ch0_end boundary, partial
    # tail chunk always its own DMA
    groups = []  # (start, nchunks)
    st = 0
    while st < kc1 - 1:
        if st < ch0_end <= st + 2 and ch0_end < kc1 - 1:
            ln = ch0_end - st
        else:
            ln = min(2, kc1 - 1 - st)
        groups.append((st, ln))
        st += ln
    groups.append((kc1 - 1, 1))  # tail chunk (rtail rows)
    WGL = 2  # k-chunks per W DMA group
    NWG = KC2 // WGL  # per e-half

    with ExitStack() as ctx:
        # d index decomposition: d = jj*128 + n*32 + dd
        feat_sb = ctx.enter_context(nc.sbuf_tensor([P, kc1, KC2, 4, 32], cdt))
        mbdt_sb = ctx.enter_context(nc.sbuf_tensor([P, kc1, 2, SH], cdt))
        w_sb = ctx.enter_context(nc.sbuf_tensor([P, ES, KC2, 512], cdt))
        bias_sb = ctx.enter_context(nc.sbuf_tensor([1, EW], cdt))
        ones_sb = ctx.enter_context(nc.sbuf_tensor([1, SH], cdt))
        agg_sb = [
            ctx.enter_context(nc.sbuf_tensor(f"agg{h}", [P, D // 4], cdt))
            for h in range(2)
        ]
        aggt_sb = [
            ctx.enter_context(nc.sbuf_tensor(f"aggt{h}", [P, KC2, SH], cdt))
            for h in range(2)
        ]
        out_sb = ctx.enter_context(nc.sbuf_tensor([P, 512], F16))
        pm = [
            ctx.enter_context(nc.psum_tensor(f"pm{h}", [P, D // 4], F32))
            for h in range(2)
        ]
        po = ctx.enter_context(nc.psum_tensor([P, 512], F32))

        fsems = [ctx.enter_context(nc.semaphore(f"fs{g}")) for g in range(len(groups))]
        wsems = [
            ctx.enter_context(nc.semaphore(f"ws{et}_{g}"))
            for et in range(ES)
            for g in range(NWG)
        ]
        csem = ctx.enter_context(nc.semaphore("csem"))
        vsem = ctx.enter_context(nc.semaphore("vsem"))
        s1_sem = [ctx.enter_context(nc.semaphore(f"s1_{h}")) for h in range(2)]
        tr_sem = ctx.enter_context(nc.semaphore("tr_sem"))
        s2_sem = [ctx.enter_context(nc.semaphore(f"s2_{et}")) for et in range(2)]
        act_sem = ctx.enter_context(nc.semaphore("act_sem"))
        osem = ctx.enter_context(nc.semaphore("osem"))
        block = ctx.enter_context(nc.Block(no_gpsimd_drain=True))

        @block.sync
        def _(sync):
            for g, (st2, ln) in enumerate(groups):
                pp = P if st2 + ln < kc1 else rtail
                sync.dma_start(
                    feat_sb[:pp, st2 : st2 + ln, :, :, :].rearrange(
                        "p c a b e -> p c (a b e)"
                    ),
                    feat[st2 * P : st2 * P + (ln - 1) * P + pp, :].rearrange(
                        "(c p) d -> p c d", p=pp
                    ),
                ).then_inc(fsems[g], 16)

        @block.scalar
        def _(scalar):
            scalar.dma_start(
                mbdt_sb[:].rearrange("p k h j -> p (k h j)"), mbdt[:]
            ).then_inc(csem, 16)
            scalar.dma_start(bias_sb[:], bias[:]).then_inc(csem, 16)
            # W rolls in as the feat stream drains (trigger ~2 chunks early)
            scalar.wait_ge(fsems[len(groups) - 2], 16)
            for et in range(ES):
                for g in range(NWG):
                    st2 = g * WGL
                    scalar.dma_start(
                        w_sb[:, et, st2 : st2 + WGL, :].rearrange(
                            "p k e -> p (k e)"
                        ),
                        w[
                            :,
                            (et * KC2 + st2) * 512 : (et * KC2 + st2 + WGL) * 512,
                        ],
                    ).then_inc(wsems[et * NWG + g], 16)
            for et in range(ES):
                scalar.wait_ge(s2_sem[et], 1)
                nc.scalar.activation(
                    out_sb[et * SC : (et + 1) * SC, :],
                    po[et * SC : (et + 1) * SC, :],
                    mybir.ActivationFunctionType.Tanh,
                ).then_inc(act_sem, 1)
                scalar.wait_ge(act_sem, et + 1)
                scalar.dma_start(
                    out[:, et * 512 : (et + 1) * 512],
                    out_sb[et * SC : (et + 1) * SC, :],
                ).then_inc(osem, 16)
            scalar.wait_ge(osem, 32)

        @block.vector
        def _(vector):
            nc.vector.memset(ones_sb[:], 1.0).then_inc(vsem, 1)
            for h in range(2):
                vector.wait_ge(s1_sem[h], 1)
                nc.vector.tensor_copy(agg_sb[h][:], pm[h][:])
                nc.vector.drain()
                nc.vector.transpose(
                    aggt_sb[h][:].rearrange("p k j -> p (k j)"), agg_sb[h][:]
                ).then_inc(tr_sem, 1)

        @block.tensor
        def _(tensor):
            tensor.wait_ge(csem, 16)  # mbdt resident
            last = [None, None]
            for g, (st2, ln) in enumerate(groups):
                tensor.wait_ge(fsems[g], 16)
                for ci in range(ln):
                    kc = st2 + ci
                    pp = P if kc < kc1 - 1 else rtail
                    for h in range(2):
                        if h == 0 and kc >= ch0_end:
                            continue
                        if h == 1 and kc < ch1_start:
                            continue
                        first = kc == (0 if h == 0 else ch1_start)
                        final = kc == ((ch0_end - 1) if h == 0 else (kc1 - 1))
                        for n in range(4):
                            last[h] = nc.tensor.matmul(
                                pm[h][n * SH : (n + 1) * SH, :],
                                mbdt_sb[:pp, kc, h, :],
                                feat_sb[:pp, kc, :, n, :],
                                start=first,
                                stop=final,
                                tile_position=(0, n * SH),
                                skip_group_check=True,
                            )
                if st2 + ln == ch0_end:
                    last[0].then_inc(s1_sem[0], 1)
            if not any(st2 + ln == ch0_end for st2, ln in groups):
                last[0].then_inc(s1_sem[0], 1)
            last[1].then_inc(s1_sem[1], 1)

            # stage 2: bias first (opens accumulation) as rank-1 matmul
            tensor.wait_ge(csem, 32)
            tensor.wait_ge(vsem, 1)
            for u in range(4):
                et, sh = divmod(u, 2)
                nc.tensor.matmul(
                    po[u * SH : (u + 1) * SH, :],
                    ones_sb[:],
                    bias_sb[:, et * 512 : (et + 1) * 512],
                    start=True,
                    stop=False,
                    tile_position=(0, u * SH),
                    skip_group_check=True,
                )
            tensor.wait_ge(tr_sem, 2)
            for et in range(ES):
                lastb = None
                for g in range(NWG):
                    tensor.wait_ge(wsems[et * NWG + g], 16)
                    for kk in range(WGL):
                        k = g * WGL + kk
                        for sh in range(2):
                            u = et * 2 + sh
                            lastb = nc.tensor.matmul(
                                po[u * SH : (u + 1) * SH, :],
                                aggt_sb[sh][:, k, :],
                                w_sb[:, et, k, :],
                                start=False,
                                stop=(k == KC2 - 1),
                                tile_position=(0, u * SH),
                                skip_group_check=True,
                            )
                lastb.then_inc(s2_sem[et], 1)

    nc.compile()
    return nc


def _host_prep(inputs: dict, compute_dtype: str):
    np_cdt = {"f32": np.float32, "bf16": None, "fp16": np.float16}[compute_dtype]
    if np_cdt is None:
        import ml_dtypes

        np_cdt = ml_dtypes.bfloat16

    feat = np.asarray(inputs["attribute_feat"])
    label = np.asarray(inputs["attribute_label"])
    w = np.asarray(inputs["W"], dtype=np.float32)
    b = np.asarray(inputs["b"], dtype=np.float32).reshape(1, D)
    mask = label > 0  # [B, A] bool

    counts = [int(mask[i * SC : (i + 1) * SC].sum()) for i in range(GS)]
    half0 = [int(mask[i * SC : i * SC + SH].sum()) for i in range(GS)]
    rmax = max(counts)
    rtot = max(64, ((rmax + 63) // 64) * 64)
    kc1 = (rtot + P - 1) // P
    rtail = rtot - (kc1 - 1) * P
    ch0_end = max((r + P - 1) // P for r in half0)
    ch1_start = min(r // P for r in half0)
    cfg = (kc1, rtail, ch0_end, ch1_start)

    gfeat, gmbd = [], []
    for i in range(GS):
        m_i = mask[i * SC : (i + 1) * SC]  # [SC, A]
        feat_i = feat[i * SC : (i + 1) * SC]
        s_idx, a_idx = np.nonzero(m_i)
        r = len(s_idx)
        fc = np.zeros((rtot, D), np_cdt)
        fc[:r] = feat_i[s_idx, a_idx].astype(np_cdt)
        mbd = np.zeros((P, kc1, 2, SH), np.float32)
        rows = np.arange(r)
        mbd[rows % P, rows // P, s_idx // SH, s_idx % SH] = 1.0
        gfeat.append(fc)
        gmbd.append(mbd.reshape(P, kc1 * SC).astype(np_cdt))

    gw, gb = [], []
    for j in range(ES):
        wj = w[:, j * EW : (j + 1) * EW]  # [D, EW]
        # layout [p, (et, k, e')]: et = 512-wide half of this core's EW
        wp = (
            wj.reshape(KC2, P, 2, 512)
            .transpose(1, 2, 0, 3)
            .reshape(P, 2 * KC2 * 512)
        )
        gw.append(np.ascontiguousarray(wp).astype(np_cdt))
        gb.append(b[:, j * EW : (j + 1) * EW].astype(np_cdt))

    in_maps = []
    for c in range(NCORES):
        i, j = divmod(c, ES)
        in_maps.append(
            {"feat": gfeat[i], "mbdt": gmbd[i], "w": gw[j], "bias": gb[j]}
        )
    return in_maps, cfg


_NC_CACHE: dict = {}


def run(inputs: dict, compute_dtype: str = COMPUTE_DTYPE, trace: bool = False):
    from concourse.bass_utils import run_bass_kernel_spmd

    in_maps, cfg = _host_prep(inputs, compute_dtype)
    key = (compute_dtype, cfg)
    if key not in _NC_CACHE:
        _NC_CACHE[key] = build_nc(cfg, compute_dtype)
    nc = _NC_CACHE[key]
    res = run_bass_kernel_spmd(nc, in_maps, list(range(NCORES)), trace=trace)
    rows = []
    for i in range(GS):
        rows.append(
            np.concatenate(
                [
                    res.results[i * ES + j]["out"].astype(np.float32)
                    for j in range(ES)
                ],
                axis=1,
            )
        )
    out = np.concatenate(rows, axis=0)
    return out, res


def kernel(**inputs) -> np.ndarray:
    out, _ = run(inputs)
    return out
